# revision 10
# baseline (speedup 1.0000x reference)
"""ArcFace (AngularPenaltySMLoss) on 8 TRN2 NeuronCores.

Strategy (model-parallel softmax sharding):
  - Shard the 32768 classes across 8 cores (4096 classes each).
  - Host prep (layout only): transpose features -> fT [512, 2048] bf16,
    transpose each weight shard -> wT [512, 4096] bf16, gather target rows
    wtgt = weight[y_true] [2048, 512] f32.
  - Device, per core:
      * normalize features: rs64[b] = 64/||f_b|| via squares + ones-matmul
        (partition-axis sum on PE), sqrt (ACT), reciprocal (DVE); broadcast
        along partitions with a K=1 matmul; fhat = fT * bcast(rs64)  [bf16]
      * normalize weight cols the same way: what = wT * bcast(1/||w_c||)
      * main loop: z[b_tile, c_chunk] = fhat.T @ what accumulated over K=512
        in PSUM; ACT Exp in place on PSUM with accum_out -> per-row partial
        exp-sums (the full exp matrix is never stored)
      * target path in natural layout (f32 exact): rawdot, ||f||^2, ||wtgt||^2
        via fused tensor_tensor_reduce; tgt = rawdot/(||f||*||wtgt||);
        numerator = 64*(t*cos(m) - sqrt(1-t^2)*sin(m))
      * ONE AllReduce (16 KB) of the per-row exp-sums across the 8 cores;
        then every core computes the identical scalar loss:
        -mean(num - log(exp(num) + fullsum - exp(64*tgt)))
"""
import math

import numpy as np
import ml_dtypes

import concourse.bass as bass
import concourse.tile as tile
from concourse import bacc, mybir
from concourse.bass_utils import run_bass_kernel_spmd

B = 2048          # batch
D = 512           # feature dim
C = 32768         # classes
NCORES = 8
CS = C // NCORES  # 4096 classes per core
S = 64.0
MARGIN = 0.5
EPS = 1e-7
COSM = math.cos(MARGIN)
SINM = math.sin(MARGIN)

NB = B // 128     # 16 batch tiles
NK = D // 128     # 4 contraction chunks
NCC = CS // 512   # 8 class chunks per core
NBC = B // 512    # 4 batch chunks (row-layout ops)

F32 = mybir.dt.float32
BF16 = mybir.dt.bfloat16
AF = mybir.ActivationFunctionType
ALU = mybir.AluOpType
BF16NP = ml_dtypes.bfloat16

_CACHE = {}


def _build(use_collective=True, inplace_exp=True, main_loop=True):
    nc = bacc.Bacc(None, target_bir_lowering=False, debug=False)

    fT_ext = nc.declare_dram_parameter("fT", [D, B], BF16, isOutput=False)
    wT_ext = nc.declare_dram_parameter("wT", [D, CS], BF16, isOutput=False)
    fnat_ext = nc.declare_dram_parameter("fnat", [B, D], F32, isOutput=False)
    wtgt_ext = nc.declare_dram_parameter("wtgt", [B, D], F32, isOutput=False)
    out_ext = nc.declare_dram_parameter("out", [1, 1], F32, isOutput=True)

    cc_in = nc.dram_tensor("cc_in", [128, 2 * NB], F32)
    cc_out = nc.dram_tensor("cc_out", [128, 2 * NB], F32, addr_space="Shared")

    with tile.TileContext(nc) as tc:
        with (
            tc.tile_pool(name="persist", bufs=1) as pp,
            tc.tile_pool(name="stream", bufs=4) as sp,
        ):
            # ---- persistent SBUF tiles ----
            wt3 = pp.tile([128, NK, CS], BF16)     # raw wT   (32 KB/part)
            what3 = pp.tile([128, NK, CS], BF16)   # normalized wT
            ft3 = pp.tile([128, NK, B], BF16)      # raw fT   (16 KB/part)
            fhat3 = pp.tile([128, NK, B], BF16)    # 64 * normalized fT
            bcrn = pp.tile([128, CS], BF16)        # bcast 1/||w_c||
            bcrs = pp.tile([128, B], BF16)         # bcast 64/||f_b||
            rn_row = pp.tile([1, CS], F32)
            rs_row = pp.tile([1, B], F32)
            ones_bf = pp.tile([128, 1], BF16)
            ones_f32 = pp.tile([128, 1], F32)
            ones_row = pp.tile([1, 128], F32)
            sums2 = pp.tile([128, 2 * NB], F32)    # per-(btile, half) exp sums
            rawdot = pp.tile([128, NB], F32)
            ssf = pp.tile([128, NB], F32)
            wn2 = pp.tile([128, NB], F32)

            # ---- DMA the big operands in ----
            nc.sync.dma_start(
                wt3[:], wT_ext[:].rearrange("(k p) c -> p k c", p=128))
            nc.sync.dma_start(
                ft3[:], fT_ext[:].rearrange("(k p) b -> p k b", p=128))

            nc.vector.memset(ones_bf[:], 1.0)
            nc.vector.memset(ones_f32[:], 1.0)
            nc.vector.memset(ones_row[:], 1.0)

            # ---- target path (natural layout, f32 exact), streamed ----
            for t in range(NB):
                fn = sp.tile([128, D], F32, tag="fnat")
                nc.sync.dma_start(fn[:], fnat_ext[bass.ts(t, 128), :])
                wg = sp.tile([128, D], F32, tag="wtgtn")
                nc.sync.dma_start(wg[:], wtgt_ext[bass.ts(t, 128), :])
                prod = sp.tile([128, D], F32, tag="prod")
                nc.vector.tensor_mul(prod[:], fn[:], wg[:])
                nc.vector.reduce_sum(rawdot[:, t:t + 1], prod[:],
                                     axis=mybir.AxisListType.X)
                sq1 = sp.tile([128, D], BF16, tag="sqscr")
                nc.scalar.activation(sq1[:], fn[:], AF.Square,
                                     accum_out=ssf[:, t:t + 1])
                sq2 = sp.tile([128, D], BF16, tag="sqscr")
                nc.scalar.activation(sq2[:], wg[:], AF.Square,
                                     accum_out=wn2[:, t:t + 1])

            with tc.tile_pool(name="psmall", bufs=2, space="PSUM") as psml:
                # ---- feature norms in row layout: rs_row = 64/||f_b|| ----
                for n in range(NBC):
                    ps = psml.tile([1, 512], F32, tag="rowsum")
                    for k in range(NK):
                        fsq = sp.tile([128, 512], BF16, tag="sqt")
                        nc.vector.tensor_mul(
                            fsq[:], ft3[:, k, bass.ts(n, 512)],
                            ft3[:, k, bass.ts(n, 512)])
                        nc.tensor.matmul(
                            ps[:], ones_bf[:], fsq[:],
                            start=(k == 0), stop=(k == NK - 1))
                    # sqrt(ss/4096) = ||f||/64 ; reciprocal -> 64/||f||
                    tmp = sp.tile([1, 512], F32, tag="rowtmp")
                    nc.scalar.activation(tmp[:], ps[:], AF.Sqrt,
                                         scale=1.0 / 4096.0)
                    nc.vector.reciprocal(rs_row[:, bass.ts(n, 512)], tmp[:])
                for n in range(NBC):
                    pb = psml.tile([128, 512], F32, tag="bcast")
                    nc.tensor.matmul(pb[:], ones_row[:],
                                     rs_row[:, bass.ts(n, 512)],
                                     start=True, stop=True)
                    nc.vector.tensor_copy(bcrs[:, bass.ts(n, 512)], pb[:])
                for k in range(NK):
                    nc.vector.tensor_mul(fhat3[:, k, :], ft3[:, k, :], bcrs[:])

                # ---- weight norms in row layout: rn_row = 1/||w_c|| ----
                for n in range(NCC):
                    ps = psml.tile([1, 512], F32, tag="rowsum")
                    for k in range(NK):
                        wsq = sp.tile([128, 512], BF16, tag="sqt")
                        nc.vector.tensor_mul(
                            wsq[:], wt3[:, k, bass.ts(n, 512)],
                            wt3[:, k, bass.ts(n, 512)])
                        nc.tensor.matmul(
                            ps[:], ones_bf[:], wsq[:],
                            start=(k == 0), stop=(k == NK - 1))
                    tmp = sp.tile([1, 512], F32, tag="rowtmp")
                    nc.scalar.activation(tmp[:], ps[:], AF.Sqrt)
                    nc.vector.reciprocal(rn_row[:, bass.ts(n, 512)], tmp[:])
                for n in range(NCC):
                    pb = psml.tile([128, 512], F32, tag="bcast")
                    nc.tensor.matmul(pb[:], ones_row[:],
                                     rn_row[:, bass.ts(n, 512)],
                                     start=True, stop=True)
                    nc.vector.tensor_copy(bcrn[:, bass.ts(n, 512)], pb[:])
                for k in range(NK):
                    nc.vector.tensor_mul(what3[:, k, :], wt3[:, k, :], bcrn[:])

            # ---- combine prep (pre-AllReduce), all [128, NB] f32 ----
            rsf = pp.tile([128, NB], F32)
            rwn = pp.tile([128, NB], F32)
            tmp1 = pp.tile([128, NB], F32)
            nc.scalar.activation(tmp1[:], ssf[:], AF.Sqrt)
            nc.vector.reciprocal(rsf[:], tmp1[:])
            tmp2 = pp.tile([128, NB], F32)
            nc.scalar.activation(tmp2[:], wn2[:], AF.Sqrt)
            nc.vector.reciprocal(rwn[:], tmp2[:])
            tgt = pp.tile([128, NB], F32)
            nc.vector.tensor_mul(tgt[:], rawdot[:], rsf[:])
            nc.vector.tensor_mul(tgt[:], tgt[:], rwn[:])
            tclip = pp.tile([128, NB], F32)
            nc.vector.tensor_scalar(
                tclip[:], tgt[:], -1.0 + EPS, 1.0 - EPS,
                op0=ALU.max, op1=ALU.min)
            om = pp.tile([128, NB], F32)
            nc.vector.tensor_mul(om[:], tclip[:], tclip[:])
            nc.vector.tensor_scalar(om[:], om[:], -1.0, 1.0,
                                    op0=ALU.mult, op1=ALU.add)
            snt = pp.tile([128, NB], F32)
            nc.scalar.activation(snt[:], om[:], AF.Sqrt)
            num = pp.tile([128, NB], F32)
            nc.vector.tensor_scalar_mul(num[:], tclip[:], S * COSM)
            snts = pp.tile([128, NB], F32)
            nc.vector.tensor_scalar_mul(snts[:], snt[:], S * SINM)
            nc.vector.tensor_sub(num[:], num[:], snts[:])
            expnum = pp.tile([128, NB], F32)
            nc.scalar.activation(expnum[:], num[:], AF.Exp)
            exptgt = pp.tile([128, NB], F32)
            nc.scalar.activation(exptgt[:], tgt[:], AF.Exp, scale=S)

            # ---- main loop: matmul + in-place exp with accumulate ----
            HALF = CS // 2  # 2048 columns = 4 PSUM banks
            nc.vector.memset(sums2[:], 1.0)
            with tc.tile_pool(name="pmain", bufs=2, space="PSUM") as pmain:
                for b in range(NB if main_loop else 0):
                    for g in range(2):
                        zp = pmain.tile([128, HALF], F32, tag="z")
                        for c4 in range(4):
                            cc = g * 4 + c4
                            for k in range(NK):
                                nc.tensor.matmul(
                                    zp[:, bass.ts(c4, 512)],
                                    fhat3[:, k, bass.ts(b, 128)],
                                    what3[:, k, bass.ts(cc, 512)],
                                    start=(k == 0), stop=(k == NK - 1))
                        if inplace_exp:
                            nc.scalar.activation(
                                zp[:], zp[:], AF.Exp,
                                accum_out=sums2[:, 2 * b + g: 2 * b + g + 1])
                        else:
                            escr = sp.tile([128, HALF], BF16, tag="escr")
                            nc.scalar.activation(
                                escr[:], zp[:], AF.Exp,
                                accum_out=sums2[:, 2 * b + g: 2 * b + g + 1])

            # ---- AllReduce the partial exp sums ----
            fullsum2 = pp.tile([128, 2 * NB], F32)
            if use_collective:
                nc.sync.dma_start(cc_in[:], sums2[:])
                nc.gpsimd.collective_compute(
                    "AllReduce", ALU.add,
                    replica_groups=[list(range(NCORES))],
                    ins=[cc_in[:].opt()],
                    outs=[cc_out[:].opt()],
                )
                nc.sync.dma_start(fullsum2[:], cc_out[:])
            else:
                nc.sync.dma_start(cc_in[:], sums2[:])
                nc.sync.dma_start(fullsum2[:], cc_in[:])

            # ---- final combine (identical on every core) ----
            fs3 = fullsum2[:].rearrange("p (t g) -> p t g", g=2)
            fullsum = pp.tile([128, NB], F32)
            nc.vector.tensor_add(fullsum[:], fs3[:, :, 0], fs3[:, :, 1])
            denom = pp.tile([128, NB], F32)
            nc.vector.tensor_add(denom[:], expnum[:], fullsum[:])
            nc.vector.tensor_sub(denom[:], denom[:], exptgt[:])
            logd = pp.tile([128, NB], F32)
            nc.scalar.activation(logd[:], denom[:], AF.Ln)
            lvals = pp.tile([128, NB], F32)
            nc.vector.tensor_sub(lvals[:], num[:], logd[:])
            lred = pp.tile([128, 1], F32)
            nc.vector.reduce_sum(lred[:], lvals[:], axis=mybir.AxisListType.X)
            with tc.tile_pool(name="pfin", bufs=1, space="PSUM") as pfinp:
                pfin = pfinp.tile([1, 1], F32, tag="fin")
                nc.tensor.matmul(pfin[:], ones_f32[:], lred[:],
                                 start=True, stop=True)
                outv = pp.tile([1, 1], F32)
                nc.scalar.mul(outv[:], pfin[:], -1.0 / float(B))
            nc.sync.dma_start(out_ext[:], outv[:])

    nc.compile()
    return nc


def _prep_inputs(features, y_true, weight):
    features = np.asarray(features, dtype=np.float32)
    weight = np.asarray(weight, dtype=np.float32)
    y = np.asarray(y_true).astype(np.int64)

    fT = features.T.astype(BF16NP, order="C")          # [D, B]
    fnat = np.ascontiguousarray(features)              # [B, D] f32
    wtgt = np.ascontiguousarray(weight[y])             # [B, D] f32

    in_maps = []
    for i in range(NCORES):
        shard = weight[i * CS:(i + 1) * CS]            # [CS, D]
        wT = shard.T.astype(BF16NP, order="C")         # [D, CS]
        in_maps.append({"fT": fT, "wT": wT, "fnat": fnat, "wtgt": wtgt})
    return in_maps


def _run(features, y_true, weight, trace=False, **run_kwargs):
    if "nc" not in _CACHE:
        _CACHE["nc"] = _build()
    nc = _CACHE["nc"]
    in_maps = _prep_inputs(features, y_true, weight)
    res = run_bass_kernel_spmd(
        nc, in_maps, core_ids=list(range(NCORES)), trace=trace, **run_kwargs)
    out = np.asarray(res.results[0]["out"], dtype=np.float32)
    return np.float32(out.reshape(-1)[0]), res


def kernel(features, y_true, weight):
    val, _ = _run(features, y_true, weight, trace=False)
    return np.asarray(val, dtype=np.float32)


# revision 13
# speedup vs baseline: 1.2382x; 1.2382x over previous
"""ArcFace (AngularPenaltySMLoss) on 8 TRN2 NeuronCores.

Strategy (model-parallel softmax sharding):
  - Shard the 32768 classes across 8 cores (4096 classes each).
  - Host prep (layout only): transpose features -> fT [512, 2048] bf16,
    transpose each weight shard -> wT [512, 4096] bf16, gather target rows
    wtgt = weight[y_true] [2048, 512] f32.
  - Device, per core:
      * weight-col norms: squares (DVE) + ones-matmul partition-sum (PE),
        1/sqrt via exp(-0.5*ln(x)) on ACT (single table set), broadcast along
        partitions with a K=1 bf16 matmul; what = wT * bcast  [bf16]
      * feature norms likewise with 64/||f|| = exp(-0.5*ln(ss) + ln(64));
        fhat = fT * bcast  [bf16]
      * main loop: z = fhat.T @ what accumulated over K=512 in PSUM (bf16
        matmuls); ACT Exp in place on PSUM with accum_out -> per-row partial
        exp sums (the full exp matrix is never stored)
      * target path (concurrent with main loop, on GpSimd+DVE): rawdot,
        ||f||^2, ||wtgt||^2 via gpsimd mult + DVE reduce (f32 exact);
        tgt = rawdot * exp(-0.5*(ln(ssf)+ln(wn2)));
        numerator = 64*(t*cos(m) - sqrt(1-t^2)*sin(m)), sqrt via exp/ln
      * ONE AllReduce (16 KB) of the per-row exp sums across the 8 cores;
        every core computes the identical scalar loss:
        -mean(num - log(exp(num) + fullsum - exp(64*tgt)))
"""
import math

import numpy as np
import ml_dtypes

import concourse.bass as bass
import concourse.tile as tile
from concourse import bacc, mybir
from concourse.bass_utils import run_bass_kernel_spmd

B = 2048          # batch
D = 512           # feature dim
C = 32768         # classes
NCORES = 8
CS = C // NCORES  # 4096 classes per core
S = 64.0
MARGIN = 0.5
EPS = 1e-7
COSM = math.cos(MARGIN)
SINM = math.sin(MARGIN)
LN_S = math.log(S)

NB = B // 128     # 16 batch tiles
NK = D // 128     # 4 contraction chunks
NCC = CS // 512   # 8 class chunks per core
NBC = B // 512    # 4 batch chunks (row-layout ops)

F32 = mybir.dt.float32
BF16 = mybir.dt.bfloat16
AF = mybir.ActivationFunctionType
ALU = mybir.AluOpType
BF16NP = ml_dtypes.bfloat16

_CACHE = {}


def _build():
    nc = bacc.Bacc(None, target_bir_lowering=False, debug=False)

    fT_ext = nc.declare_dram_parameter("fT", [D, B], BF16, isOutput=False)
    wT_ext = nc.declare_dram_parameter("wT", [D, CS], BF16, isOutput=False)
    fnat_ext = nc.declare_dram_parameter("fnat", [B, D], F32, isOutput=False)
    wtgt_ext = nc.declare_dram_parameter("wtgt", [B, D], F32, isOutput=False)
    out_ext = nc.declare_dram_parameter("out", [1, 1], F32, isOutput=True)

    cc_in = nc.dram_tensor("cc_in", [128, 2 * NB], F32)
    cc_out = nc.dram_tensor("cc_out", [128, 2 * NB], F32, addr_space="Shared")

    with tile.TileContext(nc) as tc:
        with (
            tc.tile_pool(name="persist", bufs=1) as pp,
            tc.tile_pool(name="stream", bufs=4) as sp,
        ):
            # ---- persistent SBUF tiles ----
            wt3 = pp.tile([128, NK, CS], BF16)     # raw wT (32 KB/part)
            whats = [pp.tile([128, NK, 512], BF16, tag=f"what{i}", name=f"what{i}")
                     for i in range(NCC)]          # normalized wT, per chunk
            ft3 = pp.tile([128, NK, B], BF16)      # raw fT (16 KB/part)
            fhat3 = pp.tile([128, NK, B], BF16)    # 64 * normalized fT
            ones_bf = pp.tile([128, 1], BF16)
            inv_bf = pp.tile([128, 1], BF16)   # 1/4096: folds 64^2 into ssf
            ones_f32 = pp.tile([128, 1], F32)
            ones_row = pp.tile([1, 128], BF16)
            sums2 = pp.tile([128, 2 * NB], F32)    # per-(btile, half) exp sums
            rawdot = pp.tile([128, NB], F32)
            ssf = pp.tile([128, NB], F32)
            wn2 = pp.tile([128, NB], F32)

            # ---- DMA the matmul operands in ----
            nc.sync.dma_start(
                wt3[:], wT_ext[:].rearrange("(k p) c -> p k c", p=128))
            nc.sync.dma_start(
                ft3[:], fT_ext[:].rearrange("(k p) b -> p k b", p=128))

            nc.vector.memset(ones_bf[:], 1.0)
            nc.vector.memset(inv_bf[:], 1.0 / 4096.0)
            nc.vector.memset(ones_f32[:], 1.0)
            nc.vector.memset(ones_row[:], 1.0)

            with tc.tile_pool(name="psmall", bufs=2, space="PSUM") as psml:
                # ---- weight-col norms + normalized weight, per 512-chunk ---
                for n in range(NCC):
                    ps = psml.tile([1, 512], F32, tag="rowsum")
                    for k in range(NK):
                        wsq = sp.tile([128, 512], BF16, tag="sqt")
                        nc.vector.tensor_mul(
                            wsq[:], wt3[:, k, bass.ts(n, 512)],
                            wt3[:, k, bass.ts(n, 512)])
                        nc.tensor.matmul(
                            ps[:], ones_bf[:], wsq[:],
                            start=(k == 0), stop=(k == NK - 1))
                    lnr = sp.tile([1, 512], F32, tag="lnrow")
                    nc.scalar.activation(lnr[:], ps[:], AF.Ln)
                    rnr = sp.tile([1, 512], BF16, tag="rnrow")
                    nc.scalar.activation(rnr[:], lnr[:], AF.Exp, scale=-0.5)
                    pb = psml.tile([128, 512], F32, tag="bcast")
                    nc.tensor.matmul(pb[:], ones_row[:], rnr[:],
                                     start=True, stop=True)
                    bc = sp.tile([128, 512], BF16, tag="bc")
                    nc.vector.tensor_copy(bc[:], pb[:])
                    for k in range(NK):
                        nc.vector.tensor_mul(
                            whats[n][:, k, :], wt3[:, k, bass.ts(n, 512)],
                            bc[:])

                # ---- feature norms + 64*normalized features ----
                for n in range(NBC):
                    ps = psml.tile([1, 512], F32, tag="rowsum")
                    for k in range(NK):
                        fsq = sp.tile([128, 512], BF16, tag="sqt")
                        nc.vector.tensor_mul(
                            fsq[:], ft3[:, k, bass.ts(n, 512)],
                            ft3[:, k, bass.ts(n, 512)])
                        nc.tensor.matmul(
                            ps[:], inv_bf[:], fsq[:],
                            start=(k == 0), stop=(k == NK - 1))
                    lnr = sp.tile([1, 512], F32, tag="lnrow")
                    nc.scalar.activation(lnr[:], ps[:], AF.Ln)
                    rnr = sp.tile([1, 512], BF16, tag="rnrow")
                    # ps = ss/4096 so exp(-0.5*ln(ps)) = 64/||f||
                    nc.scalar.activation(rnr[:], lnr[:], AF.Exp, scale=-0.5)
                    pb = psml.tile([128, 512], F32, tag="bcast")
                    nc.tensor.matmul(pb[:], ones_row[:], rnr[:],
                                     start=True, stop=True)
                    bc = sp.tile([128, 512], BF16, tag="bc")
                    nc.vector.tensor_copy(bc[:], pb[:])
                    for k in range(NK):
                        nc.vector.tensor_mul(
                            fhat3[:, k, bass.ts(n, 512)],
                            ft3[:, k, bass.ts(n, 512)], bc[:])

            # ---- main loop: matmul + in-place exp with accumulate ----
            HALF = CS // 2  # 2048 columns = 4 PSUM banks
            with tc.tile_pool(name="pmain", bufs=2, space="PSUM") as pmain:
                for b in range(NB):
                    for g in range(2):
                        zp = pmain.tile([128, HALF], F32, tag="z")
                        for c4 in range(4):
                            cc = g * 4 + c4
                            for k in range(NK):
                                nc.tensor.matmul(
                                    zp[:, bass.ts(c4, 512)],
                                    fhat3[:, k, bass.ts(b, 128)],
                                    whats[cc][:, k, :],
                                    start=(k == 0), stop=(k == NK - 1))
                        nc.scalar.activation(
                            zp[:], zp[:], AF.Exp,
                            accum_out=sums2[:, 2 * b + g: 2 * b + g + 1])

            # ---- target path (concurrent with main loop; GpSimd + DVE) ----
            for t in range(NB):
                fn = sp.tile([128, D], F32, tag="fnat")
                nc.sync.dma_start(fn[:], fnat_ext[bass.ts(t, 128), :])
                wg = sp.tile([128, D], F32, tag="wtgtn")
                nc.sync.dma_start(wg[:], wtgt_ext[bass.ts(t, 128), :])
                prod = sp.tile([128, D], F32, tag="prod")
                nc.gpsimd.tensor_mul(prod[:], fn[:], wg[:])
                nc.vector.reduce_sum(rawdot[:, t:t + 1], prod[:],
                                     axis=mybir.AxisListType.X)
                sq1 = sp.tile([128, D], F32, tag="prod")
                nc.gpsimd.tensor_mul(sq1[:], fn[:], fn[:])
                nc.vector.reduce_sum(ssf[:, t:t + 1], sq1[:],
                                     axis=mybir.AxisListType.X)
                sq2 = sp.tile([128, D], F32, tag="prod")
                nc.gpsimd.tensor_mul(sq2[:], wg[:], wg[:])
                nc.vector.reduce_sum(wn2[:, t:t + 1], sq2[:],
                                     axis=mybir.AxisListType.X)

            # ---- combine prep (pre-AllReduce), all [128, NB] f32 ----
            lssf = pp.tile([128, NB], F32)
            nc.scalar.activation(lssf[:], ssf[:], AF.Ln)
            lwn2 = pp.tile([128, NB], F32)
            nc.scalar.activation(lwn2[:], wn2[:], AF.Ln)
            lsum = pp.tile([128, NB], F32)
            nc.vector.tensor_add(lsum[:], lssf[:], lwn2[:])
            rboth = pp.tile([128, NB], F32)
            nc.scalar.activation(rboth[:], lsum[:], AF.Exp, scale=-0.5)
            tgt = pp.tile([128, NB], F32)
            nc.vector.tensor_mul(tgt[:], rawdot[:], rboth[:])
            tclip = pp.tile([128, NB], F32)
            nc.vector.tensor_scalar(
                tclip[:], tgt[:], -1.0 + EPS, 1.0 - EPS,
                op0=ALU.max, op1=ALU.min)
            om = pp.tile([128, NB], F32)
            nc.vector.tensor_mul(om[:], tclip[:], tclip[:])
            nc.vector.tensor_scalar(om[:], om[:], -1.0, 1.0,
                                    op0=ALU.mult, op1=ALU.add)
            # sqrt(om) = exp(0.5*ln(om))
            lom = pp.tile([128, NB], F32)
            nc.scalar.activation(lom[:], om[:], AF.Ln)
            snt = pp.tile([128, NB], F32)
            nc.scalar.activation(snt[:], lom[:], AF.Exp, scale=0.5)
            num = pp.tile([128, NB], F32)
            nc.vector.tensor_scalar_mul(num[:], tclip[:], S * COSM)
            snts = pp.tile([128, NB], F32)
            nc.vector.tensor_scalar_mul(snts[:], snt[:], S * SINM)
            nc.vector.tensor_sub(num[:], num[:], snts[:])
            expnum = pp.tile([128, NB], F32)
            nc.scalar.activation(expnum[:], num[:], AF.Exp)
            exptgt = pp.tile([128, NB], F32)
            nc.scalar.activation(exptgt[:], tgt[:], AF.Exp, scale=S)

            # ---- AllReduce the partial exp sums ----
            fullsum2 = pp.tile([128, 2 * NB], F32)
            nc.sync.dma_start(cc_in[:], sums2[:])
            nc.gpsimd.collective_compute(
                "AllReduce", ALU.add,
                replica_groups=[list(range(NCORES))],
                ins=[cc_in[:].opt()],
                outs=[cc_out[:].opt()],
            )
            nc.sync.dma_start(fullsum2[:], cc_out[:])

            # ---- final combine (identical on every core) ----
            fs3 = fullsum2[:].rearrange("p (t g) -> p t g", g=2)
            fullsum = pp.tile([128, NB], F32)
            nc.vector.tensor_add(fullsum[:], fs3[:, :, 0], fs3[:, :, 1])
            denom = pp.tile([128, NB], F32)
            nc.vector.tensor_add(denom[:], expnum[:], fullsum[:])
            nc.vector.tensor_sub(denom[:], denom[:], exptgt[:])
            logd = pp.tile([128, NB], F32)
            nc.scalar.activation(logd[:], denom[:], AF.Ln)
            lvals = pp.tile([128, NB], F32)
            nc.vector.tensor_sub(lvals[:], num[:], logd[:])
            lred = pp.tile([128, 1], F32)
            nc.vector.reduce_sum(lred[:], lvals[:], axis=mybir.AxisListType.X)
            with tc.tile_pool(name="pfin", bufs=1, space="PSUM") as pfinp:
                pfin = pfinp.tile([1, 1], F32, tag="fin")
                nc.tensor.matmul(pfin[:], ones_f32[:], lred[:],
                                 start=True, stop=True)
                outv = pp.tile([1, 1], F32)
                nc.scalar.mul(outv[:], pfin[:], -1.0 / float(B))
            nc.sync.dma_start(out_ext[:], outv[:])

    nc.compile()
    return nc


def _prep_inputs(features, y_true, weight):
    features = np.asarray(features, dtype=np.float32)
    weight = np.asarray(weight, dtype=np.float32)
    y = np.asarray(y_true).astype(np.int64)

    fT = features.T.astype(BF16NP, order="C")          # [D, B]
    fnat = np.ascontiguousarray(features)              # [B, D] f32
    wtgt = np.ascontiguousarray(weight[y])             # [B, D] f32

    in_maps = []
    for i in range(NCORES):
        shard = weight[i * CS:(i + 1) * CS]            # [CS, D]
        wT = shard.T.astype(BF16NP, order="C")         # [D, CS]
        in_maps.append({"fT": fT, "wT": wT, "fnat": fnat, "wtgt": wtgt})
    return in_maps


def _run(features, y_true, weight, trace=False, **run_kwargs):
    if "nc" not in _CACHE:
        _CACHE["nc"] = _build()
    nc = _CACHE["nc"]
    in_maps = _prep_inputs(features, y_true, weight)
    res = run_bass_kernel_spmd(
        nc, in_maps, core_ids=list(range(NCORES)), trace=trace, **run_kwargs)
    out = np.asarray(res.results[0]["out"], dtype=np.float32)
    return np.float32(out.reshape(-1)[0]), res


def kernel(features, y_true, weight):
    val, _ = _run(features, y_true, weight, trace=False)
    return np.asarray(val, dtype=np.float32)


# revision 14
# speedup vs baseline: 1.2829x; 1.0361x over previous
"""ArcFace (AngularPenaltySMLoss) on 8 TRN2 NeuronCores.

Strategy (model-parallel softmax sharding):
  - Shard the 32768 classes across 8 cores (4096 classes each).
  - Host prep (layout only): transpose features -> fT [512, 2048] bf16,
    transpose each weight shard -> wT [512, 4096] bf16, gather target rows
    wtgt = weight[y_true] [2048, 512] f32.
  - Device, per core:
      * weight-col norms: squares (DVE) + ones-matmul partition-sum (PE),
        ACT Sqrt + DVE reciprocal_approx on rows, broadcast along partitions
        with a K=1 bf16 matmul; what = wT * bcast  [bf16, per 512-col chunk]
      * feature norms likewise, with the 1/4096 fold so the row already
        carries the ArcFace scale: fhat = 64 * normalized fT  [bf16]
      * main loop: z = fhat.T @ what accumulated over K=512 in PSUM (bf16
        matmuls); ACT Exp in place on PSUM with accum_out -> per-row partial
        exp sums (the full exp matrix is never stored)
      * target path (concurrent with main loop, on GpSimd+DVE): rawdot,
        ||f||^2, ||wtgt||^2 via gpsimd mult + DVE reduce (f32 exact)
      * the per-row exp sums AllReduce in TWO halves: the first half's
        AllReduce hides under the second half of the main loop
      * combine (ACT ops dep-gated behind the last main-loop Exp so the
        activation table isn't thrashed mid-loop):
        tgt = rawdot * exp(-0.5*ln(ssf*wn2));
        num = 64*(t*cos(m) - sqrt(1-t^2)*sin(m)) with sqrt via exp/ln;
        loss = -mean(num - ln(exp(num) + fullsum - exp(64*tgt)))
"""
import math

import numpy as np
import ml_dtypes

import concourse.bass as bass
import concourse.tile as tile
from concourse import bacc, mybir
from concourse.bass_utils import run_bass_kernel_spmd
from concourse.tile import add_dep_helper

B = 2048          # batch
D = 512           # feature dim
C = 32768         # classes
NCORES = 8
CS = C // NCORES  # 4096 classes per core
S = 64.0
MARGIN = 0.5
EPS = 1e-7
COSM = math.cos(MARGIN)
SINM = math.sin(MARGIN)

NB = B // 128     # 16 batch tiles
NK = D // 128     # 4 contraction chunks
NCC = CS // 512   # 8 class chunks per core
NBC = B // 512    # 4 batch chunks (row-layout ops)

F32 = mybir.dt.float32
BF16 = mybir.dt.bfloat16
AF = mybir.ActivationFunctionType
ALU = mybir.AluOpType
BF16NP = ml_dtypes.bfloat16

_CACHE = {}


def _build():
    nc = bacc.Bacc(None, target_bir_lowering=False, debug=False)

    fT_ext = nc.declare_dram_parameter("fT", [D, B], BF16, isOutput=False)
    wT_ext = nc.declare_dram_parameter("wT", [D, CS], BF16, isOutput=False)
    fnat_ext = nc.declare_dram_parameter("fnat", [B, D], F32, isOutput=False)
    wtgt_ext = nc.declare_dram_parameter("wtgt", [B, D], F32, isOutput=False)
    out_ext = nc.declare_dram_parameter("out", [1, 1], F32, isOutput=True)

    ccA_in = nc.dram_tensor("ccA_in", [128, NB], F32)
    ccA_out = nc.dram_tensor("ccA_out", [128, NB], F32, addr_space="Shared")
    ccB_in = nc.dram_tensor("ccB_in", [128, NB], F32)
    ccB_out = nc.dram_tensor("ccB_out", [128, NB], F32, addr_space="Shared")

    with tile.TileContext(nc) as tc:
        with (
            tc.tile_pool(name="persist", bufs=1) as pp,
            tc.tile_pool(name="stream", bufs=4) as sp,
        ):
            # ---- persistent SBUF tiles ----
            wt3 = pp.tile([128, NK, CS], BF16)     # raw wT (32 KB/part)
            whats = [pp.tile([128, NK, 512], BF16, tag=f"what{i}",
                             name=f"what{i}")
                     for i in range(NCC)]          # normalized wT, per chunk
            ft3 = pp.tile([128, NK, B], BF16)      # raw fT (16 KB/part)
            fhat3 = pp.tile([128, NK, B], BF16)    # 64 * normalized fT
            ones_bf = pp.tile([128, 1], BF16)
            inv_bf = pp.tile([128, 1], BF16)       # 1/4096: folds 64^2 in
            ones_f32 = pp.tile([128, 1], F32)
            ones_row = pp.tile([1, 128], BF16)
            sumsA = pp.tile([128, NB], F32)        # exp sums, b tiles 0-7
            sumsB = pp.tile([128, NB], F32)        # exp sums, b tiles 8-15
            rawdot = pp.tile([128, NB], F32)
            ssf = pp.tile([128, NB], F32)
            wn2 = pp.tile([128, NB], F32)

            # ---- DMA the matmul operands in, split per k-chunk ----
            wTr = wT_ext[:].rearrange("(k p) c -> p k c", p=128)
            fTr = fT_ext[:].rearrange("(k p) b -> p k b", p=128)
            for k in range(NK):
                nc.sync.dma_start(wt3[:, k, :], wTr[:, k, :])
            for k in range(NK):
                nc.sync.dma_start(ft3[:, k, :], fTr[:, k, :])

            nc.vector.memset(ones_bf[:], 1.0)
            nc.vector.memset(inv_bf[:], 1.0 / 4096.0)
            nc.vector.memset(ones_f32[:], 1.0)
            nc.vector.memset(ones_row[:], 1.0)

            def norm_chunk(psml, src3, col0, lhs_const, dst_slices):
                """rowsum -> sqrt -> recip -> bcast -> dst = src * bcast."""
                ps = psml.tile([1, 512], F32, tag="rowsum", name="ps")
                for k in range(NK):
                    sq = sp.tile([128, 512], BF16, tag="sqt", name="sq")
                    nc.vector.tensor_mul(
                        sq[:], src3[:, k, col0:col0 + 512],
                        src3[:, k, col0:col0 + 512])
                    nc.tensor.matmul(ps[:], lhs_const[:], sq[:],
                                     start=(k == 0), stop=(k == NK - 1))
                srow = sp.tile([1, 512], F32, tag="srow", name="srow")
                nc.scalar.activation(srow[:], ps[:], AF.Sqrt)
                rrow = sp.tile([1, 512], F32, tag="rrow", name="rrow")
                nc.vector.reciprocal_approx_fast(rrow[:], srow[:])
                rnr = sp.tile([1, 512], BF16, tag="rnr", name="rnr")
                nc.vector.tensor_copy(rnr[:], rrow[:])
                pb = psml.tile([128, 512], F32, tag="bcast", name="pb")
                nc.tensor.matmul(pb[:], ones_row[:], rnr[:],
                                 start=True, stop=True)
                bc = sp.tile([128, 512], BF16, tag="bc", name="bc")
                nc.vector.tensor_copy(bc[:], pb[:])
                for k, dst in dst_slices:
                    nc.vector.tensor_mul(
                        dst, src3[:, k, col0:col0 + 512], bc[:])

            with tc.tile_pool(name="psmall", bufs=2, space="PSUM") as psml:
                # weight-col norms + normalized weight, per 512-chunk
                for n in range(NCC):
                    norm_chunk(
                        psml, wt3, 512 * n, ones_bf,
                        [(k, whats[n][:, k, :]) for k in range(NK)])
                # feature norms + 64*normalized features
                for n in range(NBC):
                    norm_chunk(
                        psml, ft3, 512 * n, inv_bf,
                        [(k, fhat3[:, k, bass.ts(n, 512)]) for k in range(NK)])

            # ---- main loop: matmul + in-place exp with accumulate ----
            HALF = CS // 2  # 2048 columns = 4 PSUM banks
            last_exp = None
            with tc.tile_pool(name="pmain", bufs=2, space="PSUM") as pmain:
                for half, sums in ((0, sumsA), (1, sumsB)):
                    for bb in range(NB // 2):
                        b = half * (NB // 2) + bb
                        for g in range(2):
                            zp = pmain.tile([128, HALF], F32, tag="z",
                                            name="zp")
                            for c4 in range(4):
                                cc = g * 4 + c4
                                for k in range(NK):
                                    nc.tensor.matmul(
                                        zp[:, bass.ts(c4, 512)],
                                        fhat3[:, k, bass.ts(b, 128)],
                                        whats[cc][:, k, :],
                                        start=(k == 0), stop=(k == NK - 1))
                            last_exp = nc.scalar.activation(
                                zp[:], zp[:], AF.Exp,
                                accum_out=sums[:, 2 * bb + g: 2 * bb + g + 1])
                    if half == 0:
                        # first half's AllReduce hides under second half
                        nc.sync.dma_start(ccA_in[:], sumsA[:])
                        nc.gpsimd.collective_compute(
                            "AllReduce", ALU.add,
                            replica_groups=[list(range(NCORES))],
                            ins=[ccA_in[:].opt()],
                            outs=[ccA_out[:].opt()],
                        )

            nc.sync.dma_start(ccB_in[:], sumsB[:])
            nc.gpsimd.collective_compute(
                "AllReduce", ALU.add,
                replica_groups=[list(range(NCORES))],
                ins=[ccB_in[:].opt()],
                outs=[ccB_out[:].opt()],
            )
            fullsumA = pp.tile([128, NB], F32)
            nc.sync.dma_start(fullsumA[:], ccA_out[:])
            fullsumB = pp.tile([128, NB], F32)
            nc.sync.dma_start(fullsumB[:], ccB_out[:])

            # ---- target path (concurrent with main loop; GpSimd + DVE) ----
            for t in range(NB):
                fn = sp.tile([128, D], F32, tag="fnat", name="fn")
                nc.sync.dma_start(fn[:], fnat_ext[bass.ts(t, 128), :])
                wg = sp.tile([128, D], F32, tag="wtgtn", name="wg")
                nc.sync.dma_start(wg[:], wtgt_ext[bass.ts(t, 128), :])
                prod = sp.tile([128, D], F32, tag="prod", name="prod")
                nc.gpsimd.tensor_mul(prod[:], fn[:], wg[:])
                nc.vector.reduce_sum(rawdot[:, t:t + 1], prod[:],
                                     axis=mybir.AxisListType.X)
                sq1 = sp.tile([128, D], F32, tag="prod", name="sq1")
                nc.gpsimd.tensor_mul(sq1[:], fn[:], fn[:])
                nc.vector.reduce_sum(ssf[:, t:t + 1], sq1[:],
                                     axis=mybir.AxisListType.X)
                sq2 = sp.tile([128, D], F32, tag="prod", name="sq2")
                nc.gpsimd.tensor_mul(sq2[:], wg[:], wg[:])
                nc.vector.reduce_sum(wn2[:, t:t + 1], sq2[:],
                                     axis=mybir.AxisListType.X)

            # ---- combine: ACT ops gated behind the last main-loop Exp ----
            m2 = pp.tile([128, NB], F32)
            nc.vector.tensor_mul(m2[:], ssf[:], wn2[:])
            lm2 = pp.tile([128, NB], F32)
            ln_gate = nc.scalar.activation(lm2[:], m2[:], AF.Ln)
            add_dep_helper(ln_gate.ins, last_exp.ins,
                           reason="keep combine ACT ops after main-loop exps")
            rboth = pp.tile([128, NB], F32)
            nc.scalar.activation(rboth[:], lm2[:], AF.Exp, scale=-0.5)
            tgt = pp.tile([128, NB], F32)
            nc.vector.tensor_mul(tgt[:], rawdot[:], rboth[:])
            exptgt = pp.tile([128, NB], F32)
            nc.scalar.activation(exptgt[:], tgt[:], AF.Exp, scale=S)
            tclip = pp.tile([128, NB], F32)
            nc.vector.tensor_scalar(
                tclip[:], tgt[:], -1.0 + EPS, 1.0 - EPS,
                op0=ALU.max, op1=ALU.min)
            om = pp.tile([128, NB], F32)
            nc.vector.tensor_mul(om[:], tclip[:], tclip[:])
            nc.vector.tensor_scalar(om[:], om[:], -1.0, 1.0,
                                    op0=ALU.mult, op1=ALU.add)
            # sqrt(om) = exp(0.5*ln(om))
            lom = pp.tile([128, NB], F32)
            nc.scalar.activation(lom[:], om[:], AF.Ln)
            snt = pp.tile([128, NB], F32)
            nc.scalar.activation(snt[:], lom[:], AF.Exp, scale=0.5)
            num = pp.tile([128, NB], F32)
            nc.vector.tensor_scalar_mul(num[:], tclip[:], S * COSM)
            snts = pp.tile([128, NB], F32)
            nc.vector.tensor_scalar_mul(snts[:], snt[:], S * SINM)
            nc.vector.tensor_sub(num[:], num[:], snts[:])
            expnum = pp.tile([128, NB], F32)
            nc.scalar.activation(expnum[:], num[:], AF.Exp)

            # ---- final combine (identical on every core) ----
            fsA = fullsumA[:].rearrange("p (t g) -> p t g", g=2)
            fsB = fullsumB[:].rearrange("p (t g) -> p t g", g=2)
            fullsum = pp.tile([128, NB], F32)
            nc.vector.tensor_add(fullsum[:, 0:NB // 2],
                                 fsA[:, :, 0], fsA[:, :, 1])
            nc.vector.tensor_add(fullsum[:, NB // 2:NB],
                                 fsB[:, :, 0], fsB[:, :, 1])
            denom = pp.tile([128, NB], F32)
            nc.vector.tensor_add(denom[:], expnum[:], fullsum[:])
            nc.vector.tensor_sub(denom[:], denom[:], exptgt[:])
            logd = pp.tile([128, NB], F32)
            nc.scalar.activation(logd[:], denom[:], AF.Ln)
            lvals = pp.tile([128, NB], F32)
            nc.vector.tensor_sub(lvals[:], num[:], logd[:])
            lred = pp.tile([128, 1], F32)
            nc.vector.reduce_sum(lred[:], lvals[:], axis=mybir.AxisListType.X)
            with tc.tile_pool(name="pfin", bufs=1, space="PSUM") as pfinp:
                pfin = pfinp.tile([1, 1], F32, tag="fin")
                nc.tensor.matmul(pfin[:], ones_f32[:], lred[:],
                                 start=True, stop=True)
                outv = pp.tile([1, 1], F32)
                nc.scalar.mul(outv[:], pfin[:], -1.0 / float(B))
            nc.sync.dma_start(out_ext[:], outv[:])

    nc.compile()
    return nc


def _prep_inputs(features, y_true, weight):
    features = np.asarray(features, dtype=np.float32)
    weight = np.asarray(weight, dtype=np.float32)
    y = np.asarray(y_true).astype(np.int64)

    fT = features.T.astype(BF16NP, order="C")          # [D, B]
    fnat = np.ascontiguousarray(features)              # [B, D] f32
    wtgt = np.ascontiguousarray(weight[y])             # [B, D] f32

    in_maps = []
    for i in range(NCORES):
        shard = weight[i * CS:(i + 1) * CS]            # [CS, D]
        wT = shard.T.astype(BF16NP, order="C")         # [D, CS]
        in_maps.append({"fT": fT, "wT": wT, "fnat": fnat, "wtgt": wtgt})
    return in_maps


def _run(features, y_true, weight, trace=False, **run_kwargs):
    if "nc" not in _CACHE:
        _CACHE["nc"] = _build()
    nc = _CACHE["nc"]
    in_maps = _prep_inputs(features, y_true, weight)
    res = run_bass_kernel_spmd(
        nc, in_maps, core_ids=list(range(NCORES)), trace=trace, **run_kwargs)
    out = np.asarray(res.results[0]["out"], dtype=np.float32)
    return np.float32(out.reshape(-1)[0]), res


def kernel(features, y_true, weight):
    val, _ = _run(features, y_true, weight, trace=False)
    return np.asarray(val, dtype=np.float32)


# revision 15
# speedup vs baseline: 1.3974x; 1.0893x over previous
"""ArcFace (AngularPenaltySMLoss) on 8 TRN2 NeuronCores.

Strategy (model-parallel softmax sharding):
  - Shard the 32768 classes across 8 cores (4096 classes each).
  - Host prep (layout only): transpose features -> fT [512, 2048] bf16,
    transpose each weight shard -> wT [512, 4096] bf16, gather target rows
    wtgt = weight[y_true] [2048, 512] f32.
  - Device, per core:
      * weight-col norms: squares (DVE) + ones-matmul partition-sum (PE),
        ACT Sqrt + DVE reciprocal_approx on rows, broadcast along partitions
        with a K=1 bf16 matmul; what = wT * bcast  [bf16, per 512-col chunk]
      * feature norms likewise, with the 1/4096 fold so the row already
        carries the ArcFace scale: fhat = 64 * normalized fT  [bf16]
      * main loop: z = fhat.T @ what accumulated over K=512 in PSUM (bf16
        matmuls); ACT Exp in place on PSUM with accum_out -> per-row partial
        exp sums (the full exp matrix is never stored)
      * target path (concurrent with main loop, on GpSimd+DVE): rawdot,
        ||f||^2, ||wtgt||^2 via gpsimd mult + DVE reduce (f32 exact)
      * the per-row exp sums AllReduce in TWO halves: the first half's
        AllReduce hides under the second half of the main loop
      * combine (ACT ops dep-gated behind the last main-loop Exp so the
        activation table isn't thrashed mid-loop):
        tgt = rawdot * exp(-0.5*ln(ssf*wn2));
        num = 64*(t*cos(m) - sqrt(1-t^2)*sin(m)) with sqrt via exp/ln;
        loss = -mean(num - ln(exp(num) + fullsum - exp(64*tgt)))
"""
import math

import numpy as np
import ml_dtypes

import concourse.bass as bass
import concourse.tile as tile
from concourse import bacc, mybir
from concourse.bass_utils import run_bass_kernel_spmd
from concourse.tile import add_dep_helper

B = 2048          # batch
D = 512           # feature dim
C = 32768         # classes
NCORES = 8
CS = C // NCORES  # 4096 classes per core
S = 64.0
MARGIN = 0.5
EPS = 1e-7
COSM = math.cos(MARGIN)
SINM = math.sin(MARGIN)

NB = B // 128     # 16 batch tiles
NK = D // 128     # 4 contraction chunks
NCC = CS // 512   # 8 class chunks per core
NBC = B // 512    # 4 batch chunks (row-layout ops)

F32 = mybir.dt.float32
BF16 = mybir.dt.bfloat16
AF = mybir.ActivationFunctionType
ALU = mybir.AluOpType
BF16NP = ml_dtypes.bfloat16

_CACHE = {}


def _build():
    nc = bacc.Bacc(None, target_bir_lowering=False, debug=False)

    fT_ext = nc.declare_dram_parameter("fT", [D, B], BF16, isOutput=False)
    wT_ext = nc.declare_dram_parameter("wT", [D, CS], BF16, isOutput=False)
    fnat_ext = nc.declare_dram_parameter("fnat", [B, D], F32, isOutput=False)
    wtgt_ext = nc.declare_dram_parameter("wtgt", [B, D], F32, isOutput=False)
    out_ext = nc.declare_dram_parameter("out", [1, 1], F32, isOutput=True)

    ccA_in = nc.dram_tensor("ccA_in", [128, NB], F32)
    ccA_out = nc.dram_tensor("ccA_out", [128, NB], F32, addr_space="Shared")
    ccB_in = nc.dram_tensor("ccB_in", [128, NB], F32)
    ccB_out = nc.dram_tensor("ccB_out", [128, NB], F32, addr_space="Shared")

    with tile.TileContext(nc) as tc:
        with (
            tc.tile_pool(name="persist", bufs=1) as pp,
            tc.tile_pool(name="stream", bufs=4) as sp,
        ):
            # ---- persistent SBUF tiles ----
            wt3 = pp.tile([128, NK, CS], BF16)     # raw wT (32 KB/part)
            whats = [pp.tile([128, NK, 512], BF16, tag=f"what{i}",
                             name=f"what{i}")
                     for i in range(NCC)]          # normalized wT, per chunk
            ft3 = pp.tile([128, NK, B], BF16)      # raw fT (16 KB/part)
            fhat3 = pp.tile([128, NK, B], BF16)    # 64 * normalized fT
            ones_bf = pp.tile([128, 1], BF16)
            inv_bf = pp.tile([128, 1], BF16)       # 1/4096: folds 64^2 in
            ones_f32 = pp.tile([128, 1], F32)
            ones_row = pp.tile([1, 128], BF16)
            sumsA = pp.tile([128, NB], F32)        # exp sums, b tiles 0-7
            sumsB = pp.tile([128, NB], F32)        # exp sums, b tiles 8-15
            rawdot = pp.tile([128, NB], F32)
            ssf = pp.tile([128, NB], F32)
            wn2 = pp.tile([128, NB], F32)

            # ---- DMA the matmul operands in, split per k-chunk ----
            wTr = wT_ext[:].rearrange("(k p) c -> p k c", p=128)
            fTr = fT_ext[:].rearrange("(k p) b -> p k b", p=128)
            for k in range(NK):
                nc.sync.dma_start(wt3[:, k, :], wTr[:, k, :])
            for k in range(NK):
                nc.sync.dma_start(ft3[:, k, :], fTr[:, k, :])

            nc.vector.memset(ones_bf[:], 1.0)
            nc.vector.memset(inv_bf[:], 1.0 / 4096.0)
            nc.vector.memset(ones_f32[:], 1.0)
            nc.vector.memset(ones_row[:], 1.0)

            def norm_chunk(psml, src3, col0, lhs_const, dst_slices):
                """rowsum -> sqrt -> recip -> bcast -> dst = src * bcast."""
                ps = psml.tile([1, 512], F32, tag="rowsum", name="ps")
                for k in range(NK):
                    sq = sp.tile([128, 512], BF16, tag="sqt", name="sq")
                    nc.scalar.activation(sq[:], src3[:, k, col0:col0 + 512],
                                         AF.Square)
                    nc.tensor.matmul(ps[:], lhs_const[:], sq[:],
                                     start=(k == 0), stop=(k == NK - 1))
                srow = sp.tile([1, 512], F32, tag="srow", name="srow")
                nc.scalar.activation(srow[:], ps[:], AF.Sqrt)
                rrow = sp.tile([1, 512], F32, tag="rrow", name="rrow")
                nc.vector.reciprocal_approx_fast(rrow[:], srow[:])
                rnr = sp.tile([1, 512], BF16, tag="rnr", name="rnr")
                nc.vector.tensor_copy(rnr[:], rrow[:])
                pb = psml.tile([128, 512], F32, tag="bcast", name="pb")
                nc.tensor.matmul(pb[:], ones_row[:], rnr[:],
                                 start=True, stop=True)
                bc = sp.tile([128, 512], BF16, tag="bc", name="bc")
                nc.vector.tensor_copy(bc[:], pb[:])
                for k, dst in dst_slices:
                    nc.vector.tensor_mul(
                        dst, src3[:, k, col0:col0 + 512], bc[:])

            with tc.tile_pool(name="psmall", bufs=2, space="PSUM") as psml:
                # feature norms + 64*normalized features (main needs these
                # first)
                for n in range(NBC):
                    norm_chunk(
                        psml, ft3, 512 * n, inv_bf,
                        [(k, fhat3[:, k, bass.ts(n, 512)]) for k in range(NK)])
                # weight-col norms + normalized weight, per 512-chunk
                for n in range(NCC):
                    norm_chunk(
                        psml, wt3, 512 * n, ones_bf,
                        [(k, whats[n][:, k, :]) for k in range(NK)])

            # ---- main loop: matmul + in-place exp with accumulate ----
            HALF = CS // 2  # 2048 columns = 4 PSUM banks
            last_exp = None
            with tc.tile_pool(name="pmain", bufs=2, space="PSUM") as pmain:
                for g, sums in ((0, sumsA), (1, sumsB)):
                    for b in range(NB):
                        zp = pmain.tile([128, HALF], F32, tag="z", name="zp")
                        for c4 in range(4):
                            cc = g * 4 + c4
                            for k in range(NK):
                                nc.tensor.matmul(
                                    zp[:, bass.ts(c4, 512)],
                                    fhat3[:, k, bass.ts(b, 128)],
                                    whats[cc][:, k, :],
                                    start=(k == 0), stop=(k == NK - 1))
                        last_exp = nc.scalar.activation(
                            zp[:], zp[:], AF.Exp,
                            accum_out=sums[:, b:b + 1])
                    if g == 0:
                        # g0 sums AllReduce hides under the g1 sweep
                        nc.sync.dma_start(ccA_in[:], sumsA[:])
                        nc.gpsimd.collective_compute(
                            "AllReduce", ALU.add,
                            replica_groups=[list(range(NCORES))],
                            ins=[ccA_in[:].opt()],
                            outs=[ccA_out[:].opt()],
                        )

            nc.sync.dma_start(ccB_in[:], sumsB[:])
            nc.gpsimd.collective_compute(
                "AllReduce", ALU.add,
                replica_groups=[list(range(NCORES))],
                ins=[ccB_in[:].opt()],
                outs=[ccB_out[:].opt()],
            )
            fullsumA = pp.tile([128, NB], F32)
            nc.sync.dma_start(fullsumA[:], ccA_out[:])
            fullsumB = pp.tile([128, NB], F32)
            nc.sync.dma_start(fullsumB[:], ccB_out[:])

            # ---- target path (concurrent with main loop; GpSimd + DVE) ----
            for t in range(NB):
                fn = sp.tile([128, D], F32, tag="fnat", name="fn")
                nc.sync.dma_start(fn[:], fnat_ext[bass.ts(t, 128), :])
                wg = sp.tile([128, D], F32, tag="wtgtn", name="wg")
                nc.sync.dma_start(wg[:], wtgt_ext[bass.ts(t, 128), :])
                prod = sp.tile([128, D], F32, tag="prod", name="prod")
                nc.gpsimd.tensor_mul(prod[:], fn[:], wg[:])
                nc.vector.reduce_sum(rawdot[:, t:t + 1], prod[:],
                                     axis=mybir.AxisListType.X)
                sq1 = sp.tile([128, D], F32, tag="prod", name="sq1")
                nc.gpsimd.tensor_mul(sq1[:], fn[:], fn[:])
                nc.vector.reduce_sum(ssf[:, t:t + 1], sq1[:],
                                     axis=mybir.AxisListType.X)
                sq2 = sp.tile([128, D], F32, tag="prod", name="sq2")
                nc.gpsimd.tensor_mul(sq2[:], wg[:], wg[:])
                nc.vector.reduce_sum(wn2[:, t:t + 1], sq2[:],
                                     axis=mybir.AxisListType.X)

            # ---- combine: ACT ops gated behind the last main-loop Exp ----
            m2 = pp.tile([128, NB], F32)
            nc.vector.tensor_mul(m2[:], ssf[:], wn2[:])
            lm2 = pp.tile([128, NB], F32)
            ln_gate = nc.scalar.activation(lm2[:], m2[:], AF.Ln)
            add_dep_helper(ln_gate.ins, last_exp.ins,
                           reason="keep combine ACT ops after main-loop exps")
            rboth = pp.tile([128, NB], F32)
            nc.scalar.activation(rboth[:], lm2[:], AF.Exp, scale=-0.5)
            tgt = pp.tile([128, NB], F32)
            nc.vector.tensor_mul(tgt[:], rawdot[:], rboth[:])
            exptgt = pp.tile([128, NB], F32)
            nc.scalar.activation(exptgt[:], tgt[:], AF.Exp, scale=S)
            tclip = pp.tile([128, NB], F32)
            nc.vector.tensor_scalar(
                tclip[:], tgt[:], -1.0 + EPS, 1.0 - EPS,
                op0=ALU.max, op1=ALU.min)
            om = pp.tile([128, NB], F32)
            nc.vector.tensor_mul(om[:], tclip[:], tclip[:])
            nc.vector.tensor_scalar(om[:], om[:], -1.0, 1.0,
                                    op0=ALU.mult, op1=ALU.add)
            # sqrt(om) = exp(0.5*ln(om))
            lom = pp.tile([128, NB], F32)
            nc.scalar.activation(lom[:], om[:], AF.Ln)
            snt = pp.tile([128, NB], F32)
            nc.scalar.activation(snt[:], lom[:], AF.Exp, scale=0.5)
            num = pp.tile([128, NB], F32)
            nc.vector.tensor_scalar_mul(num[:], tclip[:], S * COSM)
            snts = pp.tile([128, NB], F32)
            nc.vector.tensor_scalar_mul(snts[:], snt[:], S * SINM)
            nc.vector.tensor_sub(num[:], num[:], snts[:])
            expnum = pp.tile([128, NB], F32)
            nc.scalar.activation(expnum[:], num[:], AF.Exp)

            # ---- final combine (identical on every core) ----
            fullsum = pp.tile([128, NB], F32)
            nc.vector.tensor_add(fullsum[:], fullsumA[:], fullsumB[:])
            denom = pp.tile([128, NB], F32)
            nc.vector.tensor_add(denom[:], expnum[:], fullsum[:])
            nc.vector.tensor_sub(denom[:], denom[:], exptgt[:])
            logd = pp.tile([128, NB], F32)
            nc.scalar.activation(logd[:], denom[:], AF.Ln)
            lvals = pp.tile([128, NB], F32)
            nc.vector.tensor_sub(lvals[:], num[:], logd[:])
            lred = pp.tile([128, 1], F32)
            nc.vector.reduce_sum(lred[:], lvals[:], axis=mybir.AxisListType.X)
            with tc.tile_pool(name="pfin", bufs=1, space="PSUM") as pfinp:
                pfin = pfinp.tile([1, 1], F32, tag="fin")
                nc.tensor.matmul(pfin[:], ones_f32[:], lred[:],
                                 start=True, stop=True)
                outv = pp.tile([1, 1], F32)
                nc.scalar.mul(outv[:], pfin[:], -1.0 / float(B))
            nc.sync.dma_start(out_ext[:], outv[:])

    nc.compile()
    return nc


def _prep_inputs(features, y_true, weight):
    features = np.asarray(features, dtype=np.float32)
    weight = np.asarray(weight, dtype=np.float32)
    y = np.asarray(y_true).astype(np.int64)

    fT = features.T.astype(BF16NP, order="C")          # [D, B]
    fnat = np.ascontiguousarray(features)              # [B, D] f32
    wtgt = np.ascontiguousarray(weight[y])             # [B, D] f32

    in_maps = []
    for i in range(NCORES):
        shard = weight[i * CS:(i + 1) * CS]            # [CS, D]
        wT = shard.T.astype(BF16NP, order="C")         # [D, CS]
        in_maps.append({"fT": fT, "wT": wT, "fnat": fnat, "wtgt": wtgt})
    return in_maps


def _run(features, y_true, weight, trace=False, **run_kwargs):
    if "nc" not in _CACHE:
        _CACHE["nc"] = _build()
    nc = _CACHE["nc"]
    in_maps = _prep_inputs(features, y_true, weight)
    res = run_bass_kernel_spmd(
        nc, in_maps, core_ids=list(range(NCORES)), trace=trace, **run_kwargs)
    out = np.asarray(res.results[0]["out"], dtype=np.float32)
    return np.float32(out.reshape(-1)[0]), res


def kernel(features, y_true, weight):
    val, _ = _run(features, y_true, weight, trace=False)
    return np.asarray(val, dtype=np.float32)


# revision 16
# speedup vs baseline: 1.6902x; 1.2095x over previous
"""ArcFace (AngularPenaltySMLoss) on 8 TRN2 NeuronCores.

Strategy (model-parallel softmax sharding):
  - Shard the 32768 classes across 8 cores (4096 classes each).
  - Host prep (layout only): transpose features -> fT [512, 2048] bf16,
    transpose each weight shard -> wT [512, 4096] bf16, gather target rows
    wtgt = weight[y_true] [2048, 512] f32.
  - Device, per core:
      * weight-col norms: squares (DVE) + ones-matmul partition-sum (PE),
        ACT Sqrt + DVE reciprocal_approx on rows, broadcast along partitions
        with a K=1 bf16 matmul; what = wT * bcast  [bf16, per 512-col chunk]
      * feature norms likewise, with the 1/4096 fold so the row already
        carries the ArcFace scale: fhat = 64 * normalized fT  [bf16]
      * main loop: z = fhat.T @ what accumulated over K=512 in PSUM (bf16
        matmuls); ACT Exp in place on PSUM with accum_out -> per-row partial
        exp sums (the full exp matrix is never stored)
      * target path (concurrent with main loop, on GpSimd+DVE): rawdot,
        ||f||^2, ||wtgt||^2 via gpsimd mult + DVE reduce (f32 exact)
      * the per-row exp sums AllReduce in TWO halves: the first half's
        AllReduce hides under the second half of the main loop
      * combine (ACT ops dep-gated behind the last main-loop Exp so the
        activation table isn't thrashed mid-loop):
        tgt = rawdot * exp(-0.5*ln(ssf*wn2));
        num = 64*(t*cos(m) - sqrt(1-t^2)*sin(m)) with sqrt via exp/ln;
        loss = -mean(num - ln(exp(num) + fullsum - exp(64*tgt)))
"""
import math

import numpy as np
import ml_dtypes

import concourse.bass as bass
import concourse.tile as tile
from concourse import bacc, mybir
from concourse.bass_utils import run_bass_kernel_spmd
from concourse.tile import add_dep_helper

B = 2048          # batch
D = 512           # feature dim
C = 32768         # classes
NCORES = 8
CS = C // NCORES  # 4096 classes per core
S = 64.0
MARGIN = 0.5
EPS = 1e-7
COSM = math.cos(MARGIN)
SINM = math.sin(MARGIN)

NB = B // 128     # 16 batch tiles
NK = D // 128     # 4 contraction chunks
NCC = CS // 512   # 8 class chunks per core
NBC = B // 512    # 4 batch chunks (row-layout ops)

F32 = mybir.dt.float32
BF16 = mybir.dt.bfloat16
AF = mybir.ActivationFunctionType
ALU = mybir.AluOpType
BF16NP = ml_dtypes.bfloat16
FP8 = mybir.dt.float8e4
FP8NP = ml_dtypes.float8_e4m3fn

USE_FP8 = True
MMDT = FP8 if USE_FP8 else BF16
MMNP = FP8NP if USE_FP8 else BF16NP

_CACHE = {}


def _build():
    nc = bacc.Bacc(None, target_bir_lowering=False, debug=False)

    fT_ext = nc.declare_dram_parameter("fT", [D, B], MMDT, isOutput=False)
    wT_ext = nc.declare_dram_parameter("wT", [D, CS], MMDT, isOutput=False)
    fnat_ext = nc.declare_dram_parameter("fnat", [B, D], F32, isOutput=False)
    wtgt_ext = nc.declare_dram_parameter("wtgt", [B, D], F32, isOutput=False)
    out_ext = nc.declare_dram_parameter("out", [1, 1], F32, isOutput=True)

    ccA_in = nc.dram_tensor("ccA_in", [128, NB], F32)
    ccA_out = nc.dram_tensor("ccA_out", [128, NB], F32, addr_space="Shared")
    ccB_in = nc.dram_tensor("ccB_in", [128, NB], F32)
    ccB_out = nc.dram_tensor("ccB_out", [128, NB], F32, addr_space="Shared")

    with tile.TileContext(nc) as tc:
        with (
            tc.tile_pool(name="persist", bufs=1) as pp,
            tc.tile_pool(name="stream", bufs=4) as sp,
        ):
            # ---- persistent SBUF tiles ----
            wt3 = pp.tile([128, NK, CS], MMDT)     # raw wT
            whats = [pp.tile([128, NK, 512], MMDT, tag=f"what{i}",
                             name=f"what{i}")
                     for i in range(NCC)]          # normalized wT, per chunk
            ft3 = pp.tile([128, NK, B], MMDT)      # raw fT
            fhat3 = pp.tile([128, NK, B], MMDT)    # 64 * normalized fT
            ones_bf = pp.tile([128, 1], BF16)
            inv_bf = pp.tile([128, 1], BF16)       # 1/4096: folds 64^2 in
            ones_f32 = pp.tile([128, 1], F32)
            ones_row = pp.tile([1, 128], BF16)
            sumsA = pp.tile([128, NB], F32)        # exp sums, b tiles 0-7
            sumsB = pp.tile([128, NB], F32)        # exp sums, b tiles 8-15
            rawdot = pp.tile([128, NB], F32)
            ssf = pp.tile([128, NB], F32)
            wn2 = pp.tile([128, NB], F32)

            # ---- DMA the matmul operands in, split per k-chunk ----
            wTr = wT_ext[:].rearrange("(k p) c -> p k c", p=128)
            fTr = fT_ext[:].rearrange("(k p) b -> p k b", p=128)
            for k in range(NK):
                nc.sync.dma_start(wt3[:, k, :], wTr[:, k, :])
            for k in range(NK):
                nc.sync.dma_start(ft3[:, k, :], fTr[:, k, :])

            nc.vector.memset(ones_bf[:], 1.0)
            nc.vector.memset(inv_bf[:], 1.0 / 4096.0)
            nc.vector.memset(ones_f32[:], 1.0)
            nc.vector.memset(ones_row[:], 1.0)

            def norm_chunk(psml, src3, col0, lhs_const, dst_slices):
                """rowsum -> sqrt -> recip -> bcast -> dst = src * bcast."""
                ps = psml.tile([1, 512], F32, tag="rowsum", name="ps")
                for k in range(NK):
                    sq = sp.tile([128, 512], BF16, tag="sqt", name="sq")
                    nc.scalar.activation(sq[:], src3[:, k, col0:col0 + 512],
                                         AF.Square)
                    nc.tensor.matmul(ps[:], lhs_const[:], sq[:],
                                     start=(k == 0), stop=(k == NK - 1))
                srow = sp.tile([1, 512], F32, tag="srow", name="srow")
                nc.scalar.activation(srow[:], ps[:], AF.Sqrt)
                rrow = sp.tile([1, 512], F32, tag="rrow", name="rrow")
                nc.vector.reciprocal_approx_fast(rrow[:], srow[:])
                rnr = sp.tile([1, 512], BF16, tag="rnr", name="rnr")
                nc.vector.tensor_copy(rnr[:], rrow[:])
                pb = psml.tile([128, 512], F32, tag="bcast", name="pb")
                nc.tensor.matmul(pb[:], ones_row[:], rnr[:],
                                 start=True, stop=True)
                bc = sp.tile([128, 512], BF16, tag="bc", name="bc")
                nc.vector.tensor_copy(bc[:], pb[:])
                for k, dst in dst_slices:
                    nc.vector.tensor_mul(
                        dst, src3[:, k, col0:col0 + 512], bc[:])

            with tc.tile_pool(name="psmall", bufs=2, space="PSUM") as psml:
                # feature norms + 64*normalized features (main needs these
                # first)
                for n in range(NBC):
                    norm_chunk(
                        psml, ft3, 512 * n, inv_bf,
                        [(k, fhat3[:, k, bass.ts(n, 512)]) for k in range(NK)])
                # weight-col norms + normalized weight, per 512-chunk
                for n in range(NCC):
                    norm_chunk(
                        psml, wt3, 512 * n, ones_bf,
                        [(k, whats[n][:, k, :]) for k in range(NK)])

            # ---- main loop: matmul + in-place exp with accumulate ----
            HALF = CS // 2  # 2048 columns = 4 PSUM banks
            last_exp = None
            with tc.tile_pool(name="pmain", bufs=2, space="PSUM") as pmain:
                for g, sums in ((0, sumsA), (1, sumsB)):
                    for b in range(NB):
                        zp = pmain.tile([128, HALF], F32, tag="z", name="zp")
                        for c4 in range(4):
                            cc = g * 4 + c4
                            if USE_FP8:
                                for j in range(NK // 2):
                                    nc.tensor.matmul(
                                        zp[:, bass.ts(c4, 512)],
                                        fhat3[:, 2 * j:2 * j + 2,
                                              bass.ts(b, 128)],
                                        whats[cc][:, 2 * j:2 * j + 2, :],
                                        start=(j == 0), stop=(j == 1),
                                        perf_mode=mybir.MatmulPerfMode.DoubleRow)
                            else:
                                for k in range(NK):
                                    nc.tensor.matmul(
                                        zp[:, bass.ts(c4, 512)],
                                        fhat3[:, k, bass.ts(b, 128)],
                                        whats[cc][:, k, :],
                                        start=(k == 0), stop=(k == NK - 1))
                        last_exp = nc.scalar.activation(
                            zp[:], zp[:], AF.Exp,
                            accum_out=sums[:, b:b + 1])
                    if g == 0:
                        # g0 sums AllReduce hides under the g1 sweep
                        nc.sync.dma_start(ccA_in[:], sumsA[:])
                        nc.gpsimd.collective_compute(
                            "AllReduce", ALU.add,
                            replica_groups=[list(range(NCORES))],
                            ins=[ccA_in[:].opt()],
                            outs=[ccA_out[:].opt()],
                        )

            nc.sync.dma_start(ccB_in[:], sumsB[:])
            nc.gpsimd.collective_compute(
                "AllReduce", ALU.add,
                replica_groups=[list(range(NCORES))],
                ins=[ccB_in[:].opt()],
                outs=[ccB_out[:].opt()],
            )
            fullsumA = pp.tile([128, NB], F32)
            nc.sync.dma_start(fullsumA[:], ccA_out[:])
            fullsumB = pp.tile([128, NB], F32)
            nc.sync.dma_start(fullsumB[:], ccB_out[:])

            # ---- target path (concurrent with main loop; GpSimd + DVE) ----
            for t in range(NB):
                fn = sp.tile([128, D], F32, tag="fnat", name="fn")
                nc.sync.dma_start(fn[:], fnat_ext[bass.ts(t, 128), :])
                wg = sp.tile([128, D], F32, tag="wtgtn", name="wg")
                nc.sync.dma_start(wg[:], wtgt_ext[bass.ts(t, 128), :])
                prod = sp.tile([128, D], F32, tag="prod", name="prod")
                nc.gpsimd.tensor_mul(prod[:], fn[:], wg[:])
                nc.vector.reduce_sum(rawdot[:, t:t + 1], prod[:],
                                     axis=mybir.AxisListType.X)
                sq1 = sp.tile([128, D], F32, tag="prod", name="sq1")
                nc.gpsimd.tensor_mul(sq1[:], fn[:], fn[:])
                nc.vector.reduce_sum(ssf[:, t:t + 1], sq1[:],
                                     axis=mybir.AxisListType.X)
                sq2 = sp.tile([128, D], F32, tag="prod", name="sq2")
                nc.gpsimd.tensor_mul(sq2[:], wg[:], wg[:])
                nc.vector.reduce_sum(wn2[:, t:t + 1], sq2[:],
                                     axis=mybir.AxisListType.X)

            # ---- combine: ACT ops gated behind the last main-loop Exp ----
            m2 = pp.tile([128, NB], F32)
            nc.vector.tensor_mul(m2[:], ssf[:], wn2[:])
            lm2 = pp.tile([128, NB], F32)
            ln_gate = nc.scalar.activation(lm2[:], m2[:], AF.Ln)
            add_dep_helper(ln_gate.ins, last_exp.ins,
                           reason="keep combine ACT ops after main-loop exps")
            rboth = pp.tile([128, NB], F32)
            nc.scalar.activation(rboth[:], lm2[:], AF.Exp, scale=-0.5)
            tgt = pp.tile([128, NB], F32)
            nc.vector.tensor_mul(tgt[:], rawdot[:], rboth[:])
            exptgt = pp.tile([128, NB], F32)
            nc.scalar.activation(exptgt[:], tgt[:], AF.Exp, scale=S)
            tclip = pp.tile([128, NB], F32)
            nc.vector.tensor_scalar(
                tclip[:], tgt[:], -1.0 + EPS, 1.0 - EPS,
                op0=ALU.max, op1=ALU.min)
            om = pp.tile([128, NB], F32)
            nc.vector.tensor_mul(om[:], tclip[:], tclip[:])
            nc.vector.tensor_scalar(om[:], om[:], -1.0, 1.0,
                                    op0=ALU.mult, op1=ALU.add)
            # sqrt(om) = exp(0.5*ln(om))
            lom = pp.tile([128, NB], F32)
            nc.scalar.activation(lom[:], om[:], AF.Ln)
            snt = pp.tile([128, NB], F32)
            nc.scalar.activation(snt[:], lom[:], AF.Exp, scale=0.5)
            num = pp.tile([128, NB], F32)
            nc.vector.tensor_scalar_mul(num[:], tclip[:], S * COSM)
            snts = pp.tile([128, NB], F32)
            nc.vector.tensor_scalar_mul(snts[:], snt[:], S * SINM)
            nc.vector.tensor_sub(num[:], num[:], snts[:])
            expnum = pp.tile([128, NB], F32)
            nc.scalar.activation(expnum[:], num[:], AF.Exp)

            # ---- final combine (identical on every core) ----
            fullsum = pp.tile([128, NB], F32)
            nc.vector.tensor_add(fullsum[:], fullsumA[:], fullsumB[:])
            denom = pp.tile([128, NB], F32)
            nc.vector.tensor_add(denom[:], expnum[:], fullsum[:])
            nc.vector.tensor_sub(denom[:], denom[:], exptgt[:])
            logd = pp.tile([128, NB], F32)
            nc.scalar.activation(logd[:], denom[:], AF.Ln)
            lvals = pp.tile([128, NB], F32)
            nc.vector.tensor_sub(lvals[:], num[:], logd[:])
            lred = pp.tile([128, 1], F32)
            nc.vector.reduce_sum(lred[:], lvals[:], axis=mybir.AxisListType.X)
            with tc.tile_pool(name="pfin", bufs=1, space="PSUM") as pfinp:
                pfin = pfinp.tile([1, 1], F32, tag="fin")
                nc.tensor.matmul(pfin[:], ones_f32[:], lred[:],
                                 start=True, stop=True)
                outv = pp.tile([1, 1], F32)
                nc.scalar.mul(outv[:], pfin[:], -1.0 / float(B))
            nc.sync.dma_start(out_ext[:], outv[:])

    nc.compile()
    return nc


def _prep_inputs(features, y_true, weight):
    features = np.asarray(features, dtype=np.float32)
    weight = np.asarray(weight, dtype=np.float32)
    y = np.asarray(y_true).astype(np.int64)

    fT = features.T.astype(MMNP, order="C")            # [D, B]
    fnat = np.ascontiguousarray(features)              # [B, D] f32
    wtgt = np.ascontiguousarray(weight[y])             # [B, D] f32

    in_maps = []
    for i in range(NCORES):
        shard = weight[i * CS:(i + 1) * CS]            # [CS, D]
        wT = shard.T.astype(MMNP, order="C")           # [D, CS]
        in_maps.append({"fT": fT, "wT": wT, "fnat": fnat, "wtgt": wtgt})
    return in_maps


def _run(features, y_true, weight, trace=False, **run_kwargs):
    if "nc" not in _CACHE:
        _CACHE["nc"] = _build()
    nc = _CACHE["nc"]
    in_maps = _prep_inputs(features, y_true, weight)
    res = run_bass_kernel_spmd(
        nc, in_maps, core_ids=list(range(NCORES)), trace=trace, **run_kwargs)
    out = np.asarray(res.results[0]["out"], dtype=np.float32)
    return np.float32(out.reshape(-1)[0]), res


def kernel(features, y_true, weight):
    val, _ = _run(features, y_true, weight, trace=False)
    return np.asarray(val, dtype=np.float32)
